# revision 33
# baseline (speedup 1.0000x reference)
# Distributed causal multi-head attention kernel for one TRN2 chip (8 NeuronCores).
#
# Problem: x[2, 2048, 1024], 16 heads, head_dim 64, causal, MASK_VAL=-50000.
#   out = softmax(causal(q k^T / 8)) v @ Wo  with q = x Wq, (k|v) = x Wkv.
#
# Sharding (batch+head): core c handles batch c//4 and the 4 heads
# (c%4)*4 .. +4 (Wq/Wkv column-parallel, Wo row-parallel).  Each core writes
# a partial [2048, 1024] output; the host sums the 4 partials per batch.
# No on-device collectives.
#
# Per-core layout strategy (all bf16 compute, f32 PSUM accumulate):
#   host feeds xT = x[b].T  -> projections need no on-device transpose:
#     qT[hd,n] = Wq_shard.T @ x.T : matmul(lhsT=Wq, rhs=xT)
#     kT[hd,n] likewise; v[n,hd] = matmul(lhsT=xT, rhs=Wv)
#   scoresT[j,i] = matmul(lhsT=kT block, rhs=qT block)   (K=hd=64)
#     - even/odd heads of a pair live at partitions 0:64 / 64:128 so their
#       K=64 matmuls land in different PE row groups and run concurrently.
#   softmax: no max subtraction needed (scores ~ N(0,1); exp(-50000) == 0.0
#     in f32 exactly, matching the reference's masked softmax).  exp on ACT
#     with scale=1/8 fused.  Row sums come for free: v is augmented with a
#     ones column, so PV matmul row 64 accumulates sum_j exp.
#   causal: fully-masked j-blocks skipped; diagonal blocks compute only the
#     live column range and apply a 128x128 triangular 0/1 mask (host input).
#   out = matmul(lhsT=outT, rhs=Wo_shard), streamed out per 512-row chunk.

import os

import numpy as np
import ml_dtypes

import concourse.bass as bass
import concourse.mybir as mybir
import concourse.tile as tile
from concourse.bass_utils import run_bass_kernel_spmd


def _install_axon_ntff_shim():
    """This container's `antenv` lacks `axon_hooks`, which bass_utils imports
    when tracing under axon.  Provide the module and install the ctypes NTFF
    hook against libaxon_pjrt.so so BASS_TRACE=1 profiling works."""
    import sys
    import types
    import contextlib
    import ctypes
    try:
        import antenv.axon_hooks  # noqa: F401
        return
    except ImportError:
        pass
    try:
        import antenv
    except ImportError:
        return
    mod = types.ModuleType("antenv.axon_hooks")
    state = {"hook": None}
    mod.set_axon_ntff_profile_hook = lambda h: state.__setitem__("hook", h)
    mod.get_axon_ntff_profile_hook = lambda: state["hook"]
    sys.modules["antenv.axon_hooks"] = mod
    antenv.axon_hooks = mod
    so_path = "/opt/axon/libaxon_pjrt.so"
    try:
        lib = ctypes.CDLL(so_path)
        if not hasattr(lib, "axon_start_nrt_profile"):
            return
        lib.axon_start_nrt_profile.argtypes = [
            ctypes.POINTER(ctypes.c_int64), ctypes.c_size_t]
        lib.axon_start_nrt_profile.restype = ctypes.c_int64
        lib.axon_stop_nrt_profile.argtypes = [ctypes.c_char_p]
        lib.axon_stop_nrt_profile.restype = ctypes.c_int64

        @contextlib.contextmanager
        def _hook(output_dir, device_ids):
            import jax
            jax.devices()
            if device_ids:
                ids = (ctypes.c_int64 * len(device_ids))(*device_ids)
                rc = lib.axon_start_nrt_profile(ids, len(device_ids))
            else:
                rc = lib.axon_start_nrt_profile(None, 0)
            if rc != 0:
                raise RuntimeError(f"axon_start_nrt_profile rc={rc}")
            try:
                yield
            finally:
                n = lib.axon_stop_nrt_profile(str(output_dir).encode())
                print(f"ntff profile: {n} file(s) -> {output_dir}")

        mod.set_axon_ntff_profile_hook(_hook)
    except Exception:
        pass


_install_axon_ntff_shim()

BF16 = ml_dtypes.bfloat16
P = 128
N = 2048          # sequence length
D = 1024          # model dim
HD = 64           # head dim
HL = 4            # local heads per core
DQ = HL * HD      # 256 local projection width
KC = D // P       # 8 contraction chunks
NPAIR = HL // 2   # head pairs (even@part 0:64, odd@part 64:128)
IC = 512          # i-chunk (query) width
NIC = N // IC     # 4
NJB = N // P      # 16 j-blocks
F32 = mybir.dt.float32
BF = mybir.dt.bfloat16

LAST_RESULT = {}


def build_nc():
    nc = bass.Bass()
    xT = nc.declare_dram_parameter("xT", [D, N], BF, isOutput=False)
    wq = nc.declare_dram_parameter("wq", [D, DQ], BF, isOutput=False)
    wk = nc.declare_dram_parameter("wk", [D, DQ], BF, isOutput=False)
    wv = nc.declare_dram_parameter("wv", [D, DQ], BF, isOutput=False)
    wo = nc.declare_dram_parameter("wo", [DQ, D], BF, isOutput=False)
    mask = nc.declare_dram_parameter("mask", [P, 2, P], BF, isOutput=False)
    out = nc.declare_dram_parameter("out", [N, D], F32, isOutput=True)

    Exp = mybir.ActivationFunctionType.Exp

    with tile.TileContext(nc) as tc:
        with (
            tc.tile_pool(name="const", bufs=1) as constp,
            tc.tile_pool(name="expp", bufs=8) as expp,
            tc.tile_pool(name="normp", bufs=4) as normp,
            tc.tile_pool(name="outp", bufs=3) as outp,
            tc.tile_pool(name="psS", bufs=2, space="PSUM") as psS,
            tc.tile_pool(name="psO", bufs=1, space="PSUM") as psO,
            tc.tile_pool(name="psM", bufs=2, space="PSUM") as psM,
        ):
            # ---------------- resident SBUF tensors + input DMA ----------------
            # TRN2 has two HWDGE rings (SP + ACT).  Weights ride the SP ring,
            # xT chunks the ACT ring, interleaved so the k-th projection
            # matmul can start as soon as chunk k lands.
            xT_sb = []
            w_sbs = {}
            for name in ("wq", "wk", "wv"):
                w_sbs[name] = constp.tile([P, KC, DQ], BF, tag=name, name=name)
            wq_sb, wk_sb, wv_sb = w_sbs["wq"], w_sbs["wk"], w_sbs["wv"]
            for kc in range(KC):
                t = constp.tile([P, N], BF, tag=f"xT{kc}", name=f"xT{kc}")
                nc.scalar.dma_start(t[:], xT[kc * P:(kc + 1) * P, :])
                xT_sb.append(t)
                for name, ap in (("wq", wq), ("wk", wk), ("wv", wv)):
                    nc.sync.dma_start(
                        w_sbs[name][:, kc, :], ap[kc * P:(kc + 1) * P, :])
            wo_sb = constp.tile([P, 2, D], BF, tag="wo")
            for c in range(2):
                nc.sync.dma_start(wo_sb[:, c, :], wo[c * P:(c + 1) * P, :])
            mask_sb = constp.tile([P, 2, P], BF, tag="mask")
            nc.sync.dma_start(mask_sb[:], mask[:, :, :])

            qT_sb = constp.tile([P, NPAIR, N], BF, tag="qT")
            kT_sb = constp.tile([P, NPAIR, N], BF, tag="kT")
            # v, head-major, with a ones column appended per head (PV row 64
            # then accumulates the softmax denominator for free).
            v_sb = constp.tile([P, NJB, HL, HD + 1], BF, tag="v")
            oT_sb = constp.tile([P, NPAIR, N], BF, tag="oT")
            nc.vector.memset(v_sb[:, :, :, HD], 1.0)
            # f32 ones row at partition 64 for the reciprocal-broadcast
            # outer product (lhsT/rhs of a K=1 matmul must share a base
            # partition; the recip row lives at partition 64).  Widened to 512
            # so it can also feed PE "heater" matmuls during the DMA-paced
            # ramp (keeps the HAM clock gate at full rate).
            ones_sb = constp.tile([P, HD], BF, tag="ones")
            nc.vector.memset(ones_sb[HD:HD + 1, :], 1.0)

            # ---------------- projections ----------------
            def proj_qk(w_sb, dst, pair, i4):
                ps = psM.tile([P, IC], F32, tag="mm")
                for kc in range(KC):
                    nc.tensor.matmul(
                        ps[:],
                        w_sb[:, kc, pair * P:(pair + 1) * P],
                        xT_sb[kc][:, i4 * IC:(i4 + 1) * IC],
                        start=(kc == 0), stop=(kc == KC - 1),
                    )
                nc.vector.tensor_copy(dst[:, pair, i4 * IC:(i4 + 1) * IC], ps[:])

            def proj_v(jc):
                ps = psM.tile([P, IC], F32, tag="mm")
                for kc in range(KC):
                    nc.tensor.matmul(
                        ps[:, :DQ],
                        xT_sb[kc][:, jc * P:(jc + 1) * P],
                        wv_sb[:, kc, :],
                        start=(kc == 0), stop=(kc == KC - 1),
                    )
                nc.vector.tensor_copy(
                    v_sb[:, jc, :, 0:HD],
                    ps[:, :DQ].rearrange("p (h e) -> p h e", e=HD),
                )

            # Only i-chunk 0's projections are emitted up front; the rest are
            # sprinkled through the attention loop as PE "filler" work so the
            # PE never idles long enough for HAM to re-throttle its clock.
            for pair in range(NPAIR):
                proj_qk(wq_sb, qT_sb, pair, 0)
                proj_qk(wk_sb, kT_sb, pair, 0)
            for jc in range(4):
                proj_v(jc)

            # ---------------- attention (+ interleaved Wo) ----------------
            def wo_chunk(mc):
                # output rows mc*128..+128, all 1024 cols
                osb = outp.tile([P, D], F32, tag="osb")
                for half in range(2):
                    ps = psM.tile([P, IC], F32, tag="mm")
                    for kc2 in range(2):
                        nc.tensor.matmul(
                            ps[:],
                            oT_sb[:, kc2, mc * P:(mc + 1) * P],
                            wo_sb[:, kc2, half * IC:(half + 1) * IC],
                            start=(kc2 == 0), stop=(kc2 == 1),
                        )
                    nc.vector.tensor_copy(osb[:, half * IC:(half + 1) * IC], ps[:])
                nc.sync.dma_start(out[mc * P:(mc + 1) * P, :], osb[:])

            # Wo chunks deferred by 2 i-chunks: the late (large) i-chunks are
            # ACT-paced, so they get the Wo matmuls as PE filler density.
            wo_for_ic = {3: [0, 1, 2]}
            for i4 in range(NIC):
                # PE filler work to interleave into this i-chunk's attention.
                fillers = []
                if i4 + 1 < NIC:
                    for pair in range(NPAIR):
                        fillers.append(
                            lambda p=pair, i=i4 + 1: proj_qk(wq_sb, qT_sb, p, i))
                        fillers.append(
                            lambda p=pair, i=i4 + 1: proj_qk(wk_sb, kT_sb, p, i))
                    for jc in range(4 * (i4 + 1), 4 * (i4 + 1) + 4):
                        fillers.append(lambda j=jc: proj_v(j))
                for w4 in wo_for_ic.get(i4, []):
                    for mc in range(4 * w4, 4 * w4 + 4):
                        fillers.append(lambda m=mc: wo_chunk(m))
                fi = 0
                it = 0
                n_slots = NPAIR * (4 * i4 + 3)

                for pair in range(NPAIR):
                    po = [
                        psO.tile([HD + 1, IC], F32, tag=f"po{h01}", name=f"po{h01}")
                        for h01 in range(2)
                    ]
                    nblocks = 4 * i4 + 4
                    eTs = {}

                    def scores_exp(jb, pair=pair, i4=i4, eTs=eTs):
                        r = jb - 4 * i4  # >=0 -> diagonal block
                        lo = max(0, r * P)
                        pss = psS.tile([P, 2, IC], F32, tag="pss", name="pss")
                        for h01 in range(2):
                            pb = h01 * HD
                            nc.tensor.matmul(
                                pss[:, h01, lo:IC],
                                kT_sb[pb:pb + HD, pair, jb * P:(jb + 1) * P],
                                qT_sb[pb:pb + HD, pair, i4 * IC + lo:(i4 + 1) * IC],
                                start=True, stop=True,
                            )
                        eT = expp.tile([P, 2, IC], BF, tag="eT", name="eT")
                        nc.scalar.activation(
                            eT[:, :, lo:IC], pss[:, :, lo:IC], Exp, scale=0.125
                        )
                        if r >= 0:
                            nc.vector.tensor_mul(
                                eT[:, :, lo:lo + P], eT[:, :, lo:lo + P], mask_sb[:]
                            )
                        eTs[jb] = (eT, lo)

                    def pv(jb, pair=pair, i4=i4, po=po, nblocks=nblocks, eTs=eTs):
                        eT, lo = eTs.pop(jb)
                        for h01 in range(2):
                            nc.tensor.matmul(
                                po[h01][:, lo:IC],
                                v_sb[:, jb, 2 * pair + h01, :],
                                eT[:, h01, lo:IC],
                                start=(jb == 0), stop=(jb == nblocks - 1),
                            )

                    # software pipeline: scores(jb+1) is issued before pv(jb)
                    # so the PV never waits on a just-issued exp; filler matmul
                    # groups keep the PE dense while ACT works through exps.
                    scores_exp(0)
                    for jb in range(1, nblocks):
                        scores_exp(jb)
                        pv(jb - 1)
                        it += 1
                        # proportional pacing: spread fillers evenly over the
                        # i-chunk's attention iterations
                        while (fi < len(fillers)
                               and fi * n_slots <= it * len(fillers)):
                            fillers[fi]()
                            fi += 1
                    pv(nblocks - 1)
                    # normalize: rows 0:64 are PV, row 64 is the exp-sum.
                    # Copy the psum to SBUF right away so the po bank frees up
                    # for the next accumulation group (keeps PE from stalling
                    # into a HAM re-throttle).
                    for h01 in range(2):
                        posb = normp.tile([HD + 1, IC], F32, tag="posb")
                        nc.vector.tensor_copy(posb[:], po[h01][:])
                        # DVE reciprocal is an 8-pass iterative divide and the
                        # 512 sums live on ONE partition (~3.3us serial).
                        # Reshape them over 64 partitions via SBUF->SBUF DMA
                        # (dma_start only requires equal element counts),
                        # reciprocate in parallel lanes, and gather back.
                        rc = normp.tile([P, IC], BF, tag="rc")
                        with nc.allow_low_precision(
                                "softmax denominators are well-conditioned"):
                            if i4 == NIC - 1:
                                # tail: the two reshape DMAs (~2us completion
                                # latency each) would sit on the critical
                                # path; serial DVE recip is cheaper there.
                                nc.vector.reciprocal(
                                    rc[HD:HD + 1, :], posb[HD:HD + 1, :])
                            else:
                                sT = normp.tile([HD, 8], F32, tag="sT")
                                nc.sync.dma_start(sT[:], posb[HD:HD + 1, :])
                                rT = normp.tile([HD, 8], BF, tag="rT")
                                nc.vector.reciprocal(rT[:], sT[:])
                                nc.sync.dma_start(rc[HD:HD + 1, :], rT[:])
                        # broadcast recip row across 64 partitions via a K=1
                        # outer product on PE (shares the psM "mm" slots)
                        bc = psM.tile([HD, IC], F32, tag="mm", name="bc")
                        nc.tensor.matmul(
                            bc[:], ones_sb[HD:HD + 1, 0:HD], rc[HD:HD + 1, :],
                            start=True, stop=True,
                        )
                        if h01 == 0:
                            nc.vector.tensor_mul(
                                oT_sb[0:HD, pair, i4 * IC:(i4 + 1) * IC],
                                posb[0:HD, :], bc[:],
                            )
                        else:
                            # odd head must land at partitions 64:128 for the
                            # Wo matmul; DVE ops are kept partition-aligned and
                            # an SBUF->SBUF DMA does the partition shift.
                            ot = normp.tile([HD, IC], BF, tag="otmp")
                            nc.vector.tensor_mul(ot[:], posb[0:HD, :], bc[:])
                            nc.sync.dma_start(
                                oT_sb[HD:P, pair, i4 * IC:(i4 + 1) * IC], ot[:]
                            )
                # drain any leftover fillers for this i-chunk
                while fi < len(fillers):
                    fillers[fi]()
                    fi += 1
            # last i-chunk's Wo
            for mc in range(4 * (NIC - 1), 4 * NIC):
                wo_chunk(mc)
    return nc


_LEGALIZE_TYPES = None


def _legalize_pe_waits(nc, max_waits=1):
    """walrus' TPB instruction encodings fit very few semaphore waits
    (Matmult: 1; TensorTensor etc. similarly limited) but Tile sometimes
    emits more.  Move the excess onto an InstNoOp inserted just before the
    instruction in the same engine stream — waiting earlier on the same
    engine is always safe."""
    global _LEGALIZE_TYPES
    if _LEGALIZE_TYPES is None:
        _LEGALIZE_TYPES = (
            mybir.InstMatmult, mybir.InstLdweights, mybir.InstTensorTensor,
            mybir.InstTensorCopy, mybir.InstActivation, mybir.InstReciprocal,
            mybir.InstMemset, mybir.InstTensorReduce, mybir.InstIota,
            mybir.InstTensorScalarPtr, mybir.InstISA, mybir.InstDMACopy,
            mybir.InstTensorTensorReduce, mybir.InstDrain,
            mybir.InstDmaTransposeAnt,
        )
    n_fixed = 0
    for fn in nc.m.functions:
        for blk in fn.blocks:
            insts = list(blk.instructions)
            out = []
            for inst in insts:
                si = getattr(inst, "sync_info", None)
                if (
                    isinstance(inst, _LEGALIZE_TYPES)
                    and si is not None
                    and si.on_wait
                    and len(si.on_wait) > max_waits
                ):
                    extra = list(si.on_wait[:-max_waits])
                    keep = list(si.on_wait[-max_waits:])
                    for w in extra:
                        out.append(mybir.InstEventSemaphore(
                            name=nc.get_next_instruction_name(),
                            engine=inst.engine,
                            ins=[],
                            outs=[],
                            sync_info=mybir.SyncInfo(on_wait=[w], on_update=[]),
                            bass_nofuse=True,
                        ))
                    inst.sync_info = mybir.SyncInfo(
                        on_wait=keep, on_update=list(si.on_update)
                    )
                    n_fixed += 1
                out.append(inst)
            blk.instructions = out
    return n_fixed


_NC_CACHE = {}


def _get_nc():
    if "nc" not in _NC_CACHE:
        nc = build_nc()
        _legalize_pe_waits(nc)
        _NC_CACHE["nc"] = nc
    return _NC_CACHE["nc"]


def _make_mask():
    tri = np.triu(np.ones((P, P), np.float32))  # keep j<=c
    return np.ascontiguousarray(
        np.broadcast_to(tri[:, None, :], (P, 2, P))
    ).astype(BF16)


def kernel(x, Wq, Wkv, Wo, **kw):
    x = np.asarray(x, np.float32)
    Wq = np.asarray(Wq, np.float32)
    Wkv = np.asarray(Wkv, np.float32)
    Wo = np.asarray(Wo, np.float32)
    mask = _make_mask()

    in_maps = []
    for c in range(8):
        b = c // 4
        hs = (c % 4) * DQ
        in_maps.append({
            "xT": np.ascontiguousarray(x[b].T).astype(BF16),
            "wq": np.ascontiguousarray(Wq[:, hs:hs + DQ]).astype(BF16),
            "wk": np.ascontiguousarray(Wkv[:, hs:hs + DQ]).astype(BF16),
            "wv": np.ascontiguousarray(Wkv[:, D + hs:D + hs + DQ]).astype(BF16),
            "wo": np.ascontiguousarray(Wo[hs:hs + DQ, :]).astype(BF16),
            "mask": mask,
        })

    res = run_bass_kernel_spmd(_get_nc(), in_maps, core_ids=list(range(8)))
    LAST_RESULT["exec_time_ns"] = res.exec_time_ns
    LAST_RESULT["trace"] = res.instructions_and_trace
    parts = [np.asarray(r["out"], np.float32) for r in res.results]
    out = np.stack(
        [parts[0] + parts[1] + parts[2] + parts[3],
         parts[4] + parts[5] + parts[6] + parts[7]], axis=0
    )
    return out


# revision 34
# speedup vs baseline: 1.0312x; 1.0312x over previous
# Distributed causal multi-head attention kernel for one TRN2 chip (8 NeuronCores).
#
# Problem: x[2, 2048, 1024], 16 heads, head_dim 64, causal, MASK_VAL=-50000.
#   out = softmax(causal(q k^T / 8)) v @ Wo  with q = x Wq, (k|v) = x Wkv.
#
# Sharding (batch+head): core c handles batch c//4 and the 4 heads
# (c%4)*4 .. +4 (Wq/Wkv column-parallel, Wo row-parallel).  Each core writes
# a partial [2048, 1024] output; the host sums the 4 partials per batch.
# No on-device collectives.
#
# Per-core layout strategy (all bf16 compute, f32 PSUM accumulate):
#   host feeds xT = x[b].T  -> projections need no on-device transpose:
#     qT[hd,n] = Wq_shard.T @ x.T : matmul(lhsT=Wq, rhs=xT)
#     kT[hd,n] likewise; v[n,hd] = matmul(lhsT=xT, rhs=Wv)
#   scoresT[j,i] = matmul(lhsT=kT block, rhs=qT block)   (K=hd=64)
#     - even/odd heads of a pair live at partitions 0:64 / 64:128 so their
#       K=64 matmuls land in different PE row groups and run concurrently.
#   softmax: no max subtraction needed (scores ~ N(0,1); exp(-50000) == 0.0
#     in f32 exactly, matching the reference's masked softmax).  exp on ACT
#     with scale=1/8 fused.  Row sums come for free: v is augmented with a
#     ones column, so PV matmul row 64 accumulates sum_j exp.
#   causal: fully-masked j-blocks skipped; diagonal blocks compute only the
#     live column range and apply a 128x128 triangular 0/1 mask (host input).
#   out = matmul(lhsT=outT, rhs=Wo_shard), streamed out per 512-row chunk.

import os

import numpy as np
import ml_dtypes

import concourse.bass as bass
import concourse.mybir as mybir
import concourse.tile as tile
from concourse.bass_utils import run_bass_kernel_spmd


def _install_axon_ntff_shim():
    """This container's `antenv` lacks `axon_hooks`, which bass_utils imports
    when tracing under axon.  Provide the module and install the ctypes NTFF
    hook against libaxon_pjrt.so so BASS_TRACE=1 profiling works."""
    import sys
    import types
    import contextlib
    import ctypes
    try:
        import antenv.axon_hooks  # noqa: F401
        return
    except ImportError:
        pass
    try:
        import antenv
    except ImportError:
        return
    mod = types.ModuleType("antenv.axon_hooks")
    state = {"hook": None}
    mod.set_axon_ntff_profile_hook = lambda h: state.__setitem__("hook", h)
    mod.get_axon_ntff_profile_hook = lambda: state["hook"]
    sys.modules["antenv.axon_hooks"] = mod
    antenv.axon_hooks = mod
    so_path = "/opt/axon/libaxon_pjrt.so"
    try:
        lib = ctypes.CDLL(so_path)
        if not hasattr(lib, "axon_start_nrt_profile"):
            return
        lib.axon_start_nrt_profile.argtypes = [
            ctypes.POINTER(ctypes.c_int64), ctypes.c_size_t]
        lib.axon_start_nrt_profile.restype = ctypes.c_int64
        lib.axon_stop_nrt_profile.argtypes = [ctypes.c_char_p]
        lib.axon_stop_nrt_profile.restype = ctypes.c_int64

        @contextlib.contextmanager
        def _hook(output_dir, device_ids):
            import jax
            jax.devices()
            if device_ids:
                ids = (ctypes.c_int64 * len(device_ids))(*device_ids)
                rc = lib.axon_start_nrt_profile(ids, len(device_ids))
            else:
                rc = lib.axon_start_nrt_profile(None, 0)
            if rc != 0:
                raise RuntimeError(f"axon_start_nrt_profile rc={rc}")
            try:
                yield
            finally:
                n = lib.axon_stop_nrt_profile(str(output_dir).encode())
                print(f"ntff profile: {n} file(s) -> {output_dir}")

        mod.set_axon_ntff_profile_hook(_hook)
    except Exception:
        pass


_install_axon_ntff_shim()

BF16 = ml_dtypes.bfloat16
P = 128
N = 2048          # sequence length
D = 1024          # model dim
HD = 64           # head dim
HL = 4            # local heads per core
DQ = HL * HD      # 256 local projection width
KC = D // P       # 8 contraction chunks
NPAIR = HL // 2   # head pairs (even@part 0:64, odd@part 64:128)
IC = 512          # i-chunk (query) width
NIC = N // IC     # 4
NJB = N // P      # 16 j-blocks
F32 = mybir.dt.float32
BF = mybir.dt.bfloat16

LAST_RESULT = {}


def build_nc():
    nc = bass.Bass()
    xT = nc.declare_dram_parameter("xT", [D, N], BF, isOutput=False)
    wq = nc.declare_dram_parameter("wq", [D, DQ], BF, isOutput=False)
    wk = nc.declare_dram_parameter("wk", [D, DQ], BF, isOutput=False)
    wv = nc.declare_dram_parameter("wv", [D, DQ], BF, isOutput=False)
    wo = nc.declare_dram_parameter("wo", [DQ, D], BF, isOutput=False)
    mask = nc.declare_dram_parameter("mask", [P, 2, P], BF, isOutput=False)
    out = nc.declare_dram_parameter("out", [N, D], F32, isOutput=True)

    Exp = mybir.ActivationFunctionType.Exp

    with tile.TileContext(nc) as tc:
        with (
            tc.tile_pool(name="const", bufs=1) as constp,
            tc.tile_pool(name="expp", bufs=8) as expp,
            tc.tile_pool(name="normp", bufs=4) as normp,
            tc.tile_pool(name="outp", bufs=3) as outp,
            tc.tile_pool(name="psS", bufs=2, space="PSUM") as psS,
            tc.tile_pool(name="psO", bufs=1, space="PSUM") as psO,
            tc.tile_pool(name="psM", bufs=2, space="PSUM") as psM,
        ):
            # ---------------- resident SBUF tensors + input DMA ----------------
            # TRN2 has two HWDGE rings (SP + ACT).  Weights ride the SP ring,
            # xT chunks the ACT ring, interleaved so the k-th projection
            # matmul can start as soon as chunk k lands.
            xT_sb = []
            w_sbs = {}
            for name in ("wq", "wk", "wv"):
                w_sbs[name] = constp.tile([P, KC, DQ], BF, tag=name, name=name)
            wq_sb, wk_sb, wv_sb = w_sbs["wq"], w_sbs["wk"], w_sbs["wv"]
            for kc in range(KC):
                t = constp.tile([P, N], BF, tag=f"xT{kc}", name=f"xT{kc}")
                nc.scalar.dma_start(t[:], xT[kc * P:(kc + 1) * P, :])
                xT_sb.append(t)
                for name, ap in (("wq", wq), ("wk", wk), ("wv", wv)):
                    nc.sync.dma_start(
                        w_sbs[name][:, kc, :], ap[kc * P:(kc + 1) * P, :])
            wo_sb = constp.tile([P, 2, D], BF, tag="wo")
            for c in range(2):
                nc.sync.dma_start(wo_sb[:, c, :], wo[c * P:(c + 1) * P, :])
            mask_sb = constp.tile([P, 2, P], BF, tag="mask")
            nc.sync.dma_start(mask_sb[:], mask[:, :, :])

            qT_sb = constp.tile([P, NPAIR, N], BF, tag="qT")
            kT_sb = constp.tile([P, NPAIR, N], BF, tag="kT")
            # v, head-major, with a ones column appended per head (PV row 64
            # then accumulates the softmax denominator for free).
            v_sb = constp.tile([P, NJB, HL, HD + 1], BF, tag="v")
            oT_sb = constp.tile([P, NPAIR, N], BF, tag="oT")
            nc.vector.memset(v_sb[:, :, :, HD], 1.0)
            # f32 ones row at partition 64 for the reciprocal-broadcast
            # outer product (lhsT/rhs of a K=1 matmul must share a base
            # partition; the recip row lives at partition 64).  Widened to 512
            # so it can also feed PE "heater" matmuls during the DMA-paced
            # ramp (keeps the HAM clock gate at full rate).
            ones_sb = constp.tile([P, HD], BF, tag="ones")
            nc.vector.memset(ones_sb[HD:HD + 1, :], 1.0)

            # ---------------- projections ----------------
            def proj_qk(w_sb, dst, pair, i4):
                ps = psM.tile([P, IC], F32, tag="mm")
                for kc in range(KC):
                    nc.tensor.matmul(
                        ps[:],
                        w_sb[:, kc, pair * P:(pair + 1) * P],
                        xT_sb[kc][:, i4 * IC:(i4 + 1) * IC],
                        start=(kc == 0), stop=(kc == KC - 1),
                    )
                nc.vector.tensor_copy(dst[:, pair, i4 * IC:(i4 + 1) * IC], ps[:])

            def proj_v(jc):
                ps = psM.tile([P, IC], F32, tag="mm")
                for kc in range(KC):
                    nc.tensor.matmul(
                        ps[:, :DQ],
                        xT_sb[kc][:, jc * P:(jc + 1) * P],
                        wv_sb[:, kc, :],
                        start=(kc == 0), stop=(kc == KC - 1),
                    )
                nc.vector.tensor_copy(
                    v_sb[:, jc, :, 0:HD],
                    ps[:, :DQ].rearrange("p (h e) -> p h e", e=HD),
                )

            # Only i-chunk 0's projections are emitted up front; the rest are
            # sprinkled through the attention loop as PE "filler" work so the
            # PE never idles long enough for HAM to re-throttle its clock.
            for pair in range(NPAIR):
                proj_qk(wq_sb, qT_sb, pair, 0)
                proj_qk(wk_sb, kT_sb, pair, 0)
            for jc in range(4):
                proj_v(jc)

            # ---------------- attention (+ interleaved Wo) ----------------
            def wo_chunk(mc):
                # output rows mc*128..+128, all 1024 cols
                osb = outp.tile([P, D], F32, tag="osb")
                for half in range(2):
                    ps = psM.tile([P, IC], F32, tag="mm")
                    for kc2 in range(2):
                        nc.tensor.matmul(
                            ps[:],
                            oT_sb[:, kc2, mc * P:(mc + 1) * P],
                            wo_sb[:, kc2, half * IC:(half + 1) * IC],
                            start=(kc2 == 0), stop=(kc2 == 1),
                        )
                    nc.vector.tensor_copy(osb[:, half * IC:(half + 1) * IC], ps[:])
                nc.sync.dma_start(out[mc * P:(mc + 1) * P, :], osb[:])

            # Wo chunks deferred by 2 i-chunks: the late (large) i-chunks are
            # ACT-paced, so they get the Wo matmuls as PE filler density.
            wo_for_ic = {3: [0, 1, 2]}
            for i4 in range(NIC):
                # PE filler work to interleave into this i-chunk's attention.
                fillers = []
                if i4 + 1 < NIC:
                    for pair in range(NPAIR):
                        fillers.append(
                            lambda p=pair, i=i4 + 1: proj_qk(wq_sb, qT_sb, p, i))
                        fillers.append(
                            lambda p=pair, i=i4 + 1: proj_qk(wk_sb, kT_sb, p, i))
                    for jc in range(4 * (i4 + 1), 4 * (i4 + 1) + 4):
                        fillers.append(lambda j=jc: proj_v(j))
                for w4 in wo_for_ic.get(i4, []):
                    for mc in range(4 * w4, 4 * w4 + 4):
                        fillers.append(lambda m=mc: wo_chunk(m))
                fi = 0
                it = 0
                n_slots = NPAIR * (4 * i4 + 3)

                for pair in range(NPAIR):
                    po = [
                        psO.tile([HD + 1, IC], F32, tag=f"po{h01}", name=f"po{h01}")
                        for h01 in range(2)
                    ]
                    nblocks = 4 * i4 + 4
                    eTs = {}

                    def scores_exp(jb, pair=pair, i4=i4, eTs=eTs):
                        r = jb - 4 * i4  # >=0 -> diagonal block
                        lo = max(0, r * P)
                        pss = psS.tile([P, 2, IC], F32, tag="pss", name="pss")
                        for h01 in range(2):
                            pb = h01 * HD
                            nc.tensor.matmul(
                                pss[:, h01, lo:IC],
                                kT_sb[pb:pb + HD, pair, jb * P:(jb + 1) * P],
                                qT_sb[pb:pb + HD, pair, i4 * IC + lo:(i4 + 1) * IC],
                                start=True, stop=True,
                            )
                        eT = expp.tile([P, 2, IC], BF, tag="eT", name="eT")
                        nc.scalar.activation(
                            eT[:, :, lo:IC], pss[:, :, lo:IC], Exp, scale=0.125
                        )
                        if r >= 0:
                            nc.vector.tensor_mul(
                                eT[:, :, lo:lo + P], eT[:, :, lo:lo + P], mask_sb[:]
                            )
                        eTs[jb] = (eT, lo)

                    def pv(jb, pair=pair, i4=i4, po=po, nblocks=nblocks, eTs=eTs):
                        eT, lo = eTs.pop(jb)
                        for h01 in range(2):
                            nc.tensor.matmul(
                                po[h01][:, lo:IC],
                                v_sb[:, jb, 2 * pair + h01, :],
                                eT[:, h01, lo:IC],
                                start=(jb == 0), stop=(jb == nblocks - 1),
                            )

                    # software pipeline: scores(jb+1) is issued before pv(jb)
                    # so the PV never waits on a just-issued exp; filler matmul
                    # groups keep the PE dense while ACT works through exps.
                    scores_exp(0)
                    for jb in range(1, nblocks):
                        scores_exp(jb)
                        pv(jb - 1)
                        it += 1
                        # proportional pacing: spread fillers evenly over the
                        # i-chunk's attention iterations
                        while (fi < len(fillers)
                               and fi * n_slots <= it * len(fillers)):
                            fillers[fi]()
                            fi += 1
                    pv(nblocks - 1)
                    # normalize: rows 0:64 are PV, row 64 is the exp-sum.
                    # Copy the psum to SBUF right away so the po bank frees up
                    # for the next accumulation group (keeps PE from stalling
                    # into a HAM re-throttle).
                    for h01 in range(2):
                        posb = normp.tile([HD + 1, IC], F32, tag="posb")
                        nc.vector.tensor_copy(posb[:], po[h01][:])
                        # DVE reciprocal is an 8-pass iterative divide and the
                        # 512 sums live on ONE partition (~3.3us serial).
                        # Reshape them over 64 partitions via SBUF->SBUF DMA
                        # (dma_start only requires equal element counts),
                        # reciprocate in parallel lanes, and gather back.
                        rc = normp.tile([P, IC], BF, tag="rc")
                        with nc.allow_low_precision(
                                "softmax denominators are well-conditioned"):
                            sT = normp.tile([HD, 8], F32, tag="sT")
                            nc.sync.dma_start(sT[:], posb[HD:HD + 1, :])
                            rT = normp.tile([HD, 8], BF, tag="rT")
                            nc.vector.reciprocal(rT[:], sT[:])
                            nc.sync.dma_start(rc[HD:HD + 1, :], rT[:])
                        # broadcast recip row across 64 partitions via a K=1
                        # outer product on PE (shares the psM "mm" slots)
                        bc = psM.tile([HD, IC], F32, tag="mm", name="bc")
                        nc.tensor.matmul(
                            bc[:], ones_sb[HD:HD + 1, 0:HD], rc[HD:HD + 1, :],
                            start=True, stop=True,
                        )
                        if h01 == 0:
                            nc.vector.tensor_mul(
                                oT_sb[0:HD, pair, i4 * IC:(i4 + 1) * IC],
                                posb[0:HD, :], bc[:],
                            )
                        else:
                            # odd head must land at partitions 64:128 for the
                            # Wo matmul; DVE ops are kept partition-aligned and
                            # an SBUF->SBUF DMA does the partition shift.
                            ot = normp.tile([HD, IC], BF, tag="otmp")
                            nc.vector.tensor_mul(ot[:], posb[0:HD, :], bc[:])
                            nc.sync.dma_start(
                                oT_sb[HD:P, pair, i4 * IC:(i4 + 1) * IC], ot[:]
                            )
                # drain any leftover fillers for this i-chunk
                while fi < len(fillers):
                    fillers[fi]()
                    fi += 1
            # last i-chunk's Wo
            for mc in range(4 * (NIC - 1), 4 * NIC):
                wo_chunk(mc)
    return nc


_LEGALIZE_TYPES = None


def _legalize_pe_waits(nc, max_waits=1):
    """walrus' TPB instruction encodings fit very few semaphore waits
    (Matmult: 1; TensorTensor etc. similarly limited) but Tile sometimes
    emits more.  Move the excess onto an InstNoOp inserted just before the
    instruction in the same engine stream — waiting earlier on the same
    engine is always safe."""
    global _LEGALIZE_TYPES
    if _LEGALIZE_TYPES is None:
        _LEGALIZE_TYPES = (
            mybir.InstMatmult, mybir.InstLdweights, mybir.InstTensorTensor,
            mybir.InstTensorCopy, mybir.InstActivation, mybir.InstReciprocal,
            mybir.InstMemset, mybir.InstTensorReduce, mybir.InstIota,
            mybir.InstTensorScalarPtr, mybir.InstISA, mybir.InstDMACopy,
            mybir.InstTensorTensorReduce, mybir.InstDrain,
            mybir.InstDmaTransposeAnt,
        )
    n_fixed = 0
    for fn in nc.m.functions:
        for blk in fn.blocks:
            insts = list(blk.instructions)
            out = []
            for inst in insts:
                si = getattr(inst, "sync_info", None)
                if (
                    isinstance(inst, _LEGALIZE_TYPES)
                    and si is not None
                    and si.on_wait
                    and len(si.on_wait) > max_waits
                ):
                    extra = list(si.on_wait[:-max_waits])
                    keep = list(si.on_wait[-max_waits:])
                    for w in extra:
                        out.append(mybir.InstEventSemaphore(
                            name=nc.get_next_instruction_name(),
                            engine=inst.engine,
                            ins=[],
                            outs=[],
                            sync_info=mybir.SyncInfo(on_wait=[w], on_update=[]),
                            bass_nofuse=True,
                        ))
                    inst.sync_info = mybir.SyncInfo(
                        on_wait=keep, on_update=list(si.on_update)
                    )
                    n_fixed += 1
                out.append(inst)
            blk.instructions = out
    return n_fixed


_NC_CACHE = {}


def _get_nc():
    if "nc" not in _NC_CACHE:
        nc = build_nc()
        _legalize_pe_waits(nc)
        _NC_CACHE["nc"] = nc
    return _NC_CACHE["nc"]


def _make_mask():
    tri = np.triu(np.ones((P, P), np.float32))  # keep j<=c
    return np.ascontiguousarray(
        np.broadcast_to(tri[:, None, :], (P, 2, P))
    ).astype(BF16)


def kernel(x, Wq, Wkv, Wo, **kw):
    x = np.asarray(x, np.float32)
    Wq = np.asarray(Wq, np.float32)
    Wkv = np.asarray(Wkv, np.float32)
    Wo = np.asarray(Wo, np.float32)
    mask = _make_mask()

    in_maps = []
    for c in range(8):
        b = c // 4
        hs = (c % 4) * DQ
        in_maps.append({
            "xT": np.ascontiguousarray(x[b].T).astype(BF16),
            "wq": np.ascontiguousarray(Wq[:, hs:hs + DQ]).astype(BF16),
            "wk": np.ascontiguousarray(Wkv[:, hs:hs + DQ]).astype(BF16),
            "wv": np.ascontiguousarray(Wkv[:, D + hs:D + hs + DQ]).astype(BF16),
            "wo": np.ascontiguousarray(Wo[hs:hs + DQ, :]).astype(BF16),
            "mask": mask,
        })

    res = run_bass_kernel_spmd(_get_nc(), in_maps, core_ids=list(range(8)))
    LAST_RESULT["exec_time_ns"] = res.exec_time_ns
    LAST_RESULT["trace"] = res.instructions_and_trace
    parts = [np.asarray(r["out"], np.float32) for r in res.results]
    out = np.stack(
        [parts[0] + parts[1] + parts[2] + parts[3],
         parts[4] + parts[5] + parts[6] + parts[7]], axis=0
    )
    return out
